# revision 2
# baseline (speedup 1.0000x reference)
"""Trainium2 Bass kernel v5 for nn_DirectionalScan (2D directional diagonal-SSM + proj).

v4 = v2 (dma-transposed x, prefix-state horizontal, fp16 raw z) plus:
  - weight DMAs split and interleaved with the (mutually exclusive) xbar
    transposes so each consumer's data lands just in time
  - y_h merged into y_v on-device via a partition-permuting SWDGE
    accumulate-DMA; ONE projection for both directions (z = (y_v+y_h) @ Wp.T)
  - a16 pre-expanded to [128, (g, 64)] f16 so recurrences use fast strided APs
"""
import numpy as np
from contextlib import ExitStack

import concourse.bass as bass
import concourse.bacc as bacc
import concourse.tile as tile
from concourse import mybir
from concourse.bass_utils import run_bass_kernel_spmd
from concourse.masks import make_identity

F32 = mybir.dt.float32
F16 = mybir.dt.float16
NP_CDT = np.float16
B, H, W, D, N = 4, 64, 64, 512, 8
L, Q, CV, NOCT, NG = 64, 16, 4, 64, 32
ALU = mybir.AluOpType


def _precompute_weights(A, Bm, Cm, D_skip, Wp):
    A64, B64, C64 = A.astype(np.float64), Bm.astype(np.float64), Cm.astype(np.float64)
    CB = C64 * B64
    Apow = np.stack([A64 ** t for t in range(Q + 1)])
    Kconv = np.einsum("dn,tdn->dt", CB, Apow)
    T = np.zeros((D, Q, Q))
    for i in range(Q):
        for j in range(i + 1):
            T[:, i, j] = Kconv[:, i - j]
    T += np.eye(Q)[None] * D_skip.astype(np.float64)[:, None, None]

    W_T = np.zeros((NOCT, 128, 128))
    W_P = np.zeros((NOCT, 128, 64))
    for o in range(NOCT):
        for d8 in range(8):
            d = o * 8 + d8
            for j in range(Q):
                W_T[o, j * 8 + d8, d8::8] = T[d, :, j]
                W_P[o, j * 8 + d8, d8 * 8:d8 * 8 + 8] = Apow[Q - 1 - j, d]
    W_CBA = np.zeros((NG, 128, 256))
    for g in range(NG):
        for o2 in range(2):
            for d8 in range(8):
                d = g * 16 + o2 * 8 + d8
                for n in range(N):
                    row = o2 * 64 + d8 * 8 + n
                    W_CBA[g, row, o2 * 128 + d8:o2 * 128 + 128:8] = (
                        CB[d, n] * Apow[1:Q + 1, d, n]
                    )
    A16 = np.zeros((128, NG))
    for g in range(NG):
        for o2 in range(2):
            for d8 in range(8):
                d = g * 16 + o2 * 8 + d8
                A16[o2 * 64 + d8 * 8:o2 * 64 + d8 * 8 + 8, g] = Apow[Q, d]
    WPT = np.ascontiguousarray(Wp.astype(np.float64).T.reshape(4, 128, 512))
    wb1 = np.ascontiguousarray(
        W_P.transpose(1, 0, 2).reshape(128, NOCT * 64)).astype(NP_CDT)
    wbig = np.ascontiguousarray(np.concatenate([
        W_T.transpose(1, 0, 2).reshape(128, NOCT * 128),
        W_CBA.transpose(1, 0, 2).reshape(128, NG * 256),
    ], axis=1)).astype(NP_CDT)
    wpt = np.ascontiguousarray(
        WPT.transpose(1, 0, 2).reshape(128, 4 * 512)).astype(NP_CDT)
    return wb1, wbig, wpt, A16.astype(np.float32)


def _pack_v(x_half):
    xd = x_half.reshape(32, CV, Q, NOCT, 8)            # [s, cw, j, o, e]
    return np.ascontiguousarray(
        xd.transpose(1, 0, 3, 2, 4).reshape(128, NOCT * Q * 8)).astype(NP_CDT)


def _pack_h(x_rows):
    xd = x_rows.reshape(2, Q, W, NOCT, 8)              # [ch, j, w, o, e]
    return np.ascontiguousarray(
        xd.transpose(0, 2, 3, 1, 4).reshape(128, NOCT * Q * 8)).astype(NP_CDT)


def _prefix_states(x_pre, A):
    """Host-side: state after scanning h=0..31 down each column, without Bm.
    Returns [128 rows=(o2,d8,n), (g32, w64)] fp16 in the sp-tile layout."""
    A64 = A.astype(np.float64)
    Ap = np.stack([A64 ** (31 - j) for j in range(32)])        # [32, d, n]
    s = np.einsum("jwd,jdn->dnw", x_pre.astype(np.float64), Ap)  # [d, n, w]
    return np.ascontiguousarray(
        s.reshape(NG, 2, 8, N, W).transpose(1, 2, 3, 0, 4).reshape(128, NG * W)
    ).astype(NP_CDT)


def _xt_oct(xt, o):
    return xt[:, o * 128:(o + 1) * 128]


def _g_phase(tc, psG, w_p_sb, xt, recur, width=128):
    nc = tc.nc
    s_tiles = []
    for q in range(8):
        ps_g = psG.tile([128, 512], F32, tag="ps_g")
        for k in range(8):
            o = q * 8 + k
            hp = (o % 2) * 64
            col = (k // 2) * 128
            nc.tensor.matmul(
                ps_g[hp:hp + 64, col:col + width],
                w_p_sb[:, o * 64:o * 64 + 64], _xt_oct(xt, o)[:, 0:width],
                start=True, stop=True, skip_group_check=True,
                tile_position=(0, hp))
        s_tiles.append(recur(q, ps_g))
    return s_tiles


def _b_phase(tc, psyw, w_t_sb, w_cba_sb, xt, s_tiles, y_sb):
    nc = tc.nc
    for og in range(16):
        ps_yw = psyw.tile([128, 512], F32, tag="ps_yw")
        for oo in range(4):
            o = og * 4 + oo
            nc.tensor.matmul(ps_yw[:, oo * 128:oo * 128 + 128], _xt_oct(xt, o),
                             w_t_sb[:, o * 128:o * 128 + 128],
                             start=(oo == 0), stop=False, skip_group_check=True)
        for gg in range(2):
            g = og * 2 + gg
            s4 = s_tiles[g // 4]
            nc.tensor.matmul(ps_yw[:, gg * 256:gg * 256 + 256],
                             s4[:, (g % 4) * 128:(g % 4) * 128 + 128],
                             w_cba_sb[:, g * 256:g * 256 + 256],
                             start=False, stop=(gg == 1), skip_group_check=True)
        y_dst = y_sb[:].rearrange("p (i og o e) -> p i og o e",
                                  i=Q, og=16, o=4, e=8)[:, :, og]
        ps_src = ps_yw[:].rearrange("p (o i e) -> p i o e", o=4, i=Q, e=8)
        if og % 2 == 0:
            nc.vector.tensor_copy(y_dst, ps_src)
        else:
            nc.scalar.copy(y_dst, ps_src)


def _proj(tc, pools, wpt_sb, ident, y_sb, z_ap):
    nc = tc.nc
    psA, psout, yt_pool, out_pool = pools
    for iq in range(4):
        out_sb = out_pool.tile([128, 4 * 512], F16, tag="osb")
        for ii in range(4):
            i = iq * 4 + ii
            ps_yt = psA.tile([128, 512], F16, tag="ps_t")
            for dc in range(4):
                nc.tensor.transpose(
                    ps_yt[:, dc * 128:(dc + 1) * 128],
                    y_sb[:, i * 512 + dc * 128:i * 512 + (dc + 1) * 128], ident)
            yt = yt_pool.tile([128, 512], F16, tag="yt")
            if i % 2 == 0:
                nc.scalar.copy(yt[:], ps_yt[:])
            else:
                nc.vector.tensor_copy(yt[:], ps_yt[:])
            ps_o = psout.tile([128, 512], F32, tag="ps_o")
            for dc in range(4):
                nc.tensor.matmul(ps_o[:], yt[:, dc * 128:dc * 128 + 128],
                                 wpt_sb[:, dc * 512:dc * 512 + 512],
                                 start=(dc == 0), stop=(dc == 3))
            if i % 2 == 0:
                nc.vector.tensor_copy(out_sb[:, ii * 512:ii * 512 + 512], ps_o[:])
            else:
                nc.scalar.copy(out_sb[:, ii * 512:ii * 512 + 512], ps_o[:])
        eng = nc.sync if iq % 2 == 0 else nc.scalar
        eng.dma_start(z_ap[:, iq * 4:(iq + 1) * 4, :],
                      out_sb[:].rearrange("p (i d) -> p i d", i=4))


def _kernel_body(ctx, tc, aps):
    nc = tc.nc
    const_pool = ctx.enter_context(tc.tile_pool(name="consts", bufs=1))
    big_pool = ctx.enter_context(tc.tile_pool(name="big", bufs=1))
    s_pool = ctx.enter_context(tc.tile_pool(name="s", bufs=1))
    yt_pool = ctx.enter_context(tc.tile_pool(name="yt", bufs=2))
    out_pool = ctx.enter_context(tc.tile_pool(name="osb", bufs=2))
    psA = ctx.enter_context(tc.tile_pool(name="psA", bufs=2, space="PSUM"))
    psyw = ctx.enter_context(tc.tile_pool(name="psyw", bufs=2, space="PSUM"))
    psG = ctx.enter_context(tc.tile_pool(name="psG", bufs=2, space="PSUM"))
    psout = ctx.enter_context(tc.tile_pool(name="psout", bufs=2, space="PSUM"))

    ws_sb = const_pool.tile([128, NG + 1], F32, name="ws_sb")
    wb1_sb = const_pool.tile([128, NOCT * 64], F16, name="wb1_sb")
    wbig_sb = const_pool.tile([128, 16384], F16, name="wbig_sb")
    wpt_sb0 = const_pool.tile([128, 2048], F16, name="wpt_sb0")
    a16r_sb = const_pool.tile([128, NG * 64], F16, name="a16r_sb")
    ident = const_pool.tile([128, 128], F16, name="ident")
    a16_sb = ws_sb[:, 0:NG]
    w_p_sb = wb1_sb
    w_t_sb = wbig_sb[:, 0:8192]
    w_cba_sb = wbig_sb[:, 8192:16384]
    wpt_sb = wpt_sb0

    spre_sb = const_pool.tile([128, NG * W], F16, name="spre_sb")
    xt_h = big_pool.tile([128, NOCT * 128], F16, name="xt_h", tag="big", bufs=4)
    xt_v = big_pool.tile([128, NOCT * 128], F16, name="xt_v", tag="big", bufs=4)

    # DMA order matters: xbar transposes hold the DMA engines exclusively,
    # so interleave the plain weight loads so each lands just before use.
    nc.sync.dma_start(wb1_sb[:], aps["wb1"])
    nc.gpsimd.dma_start(ws_sb[:], aps["ws"])
    nc.gpsimd.dma_start(spre_sb[:], aps["spre"])
    nc.sync.dma_start_transpose(
        xt_h[:].rearrange("p (o f) -> p o f", o=NOCT), aps["xth"])
    nc.sync.dma_start_transpose(
        xt_v[:].rearrange("p (o f) -> p o f", o=NOCT), aps["xtv"])
    nc.gpsimd.dma_start(wbig_sb[:], aps["wbig"])
    nc.gpsimd.dma_start(wpt_sb0[:], aps["wpt"])
    make_identity(nc, ident[:])
    nc.vector.tensor_copy(
        a16r_sb[:].rearrange("p (g s) -> p g s", g=NG),
        a16_sb.rearrange("p g -> p g ()").broadcast_to((128, NG, 64)))

    def a16b(q, s):
        return a16r_sb[:, q * 256:(q + 1) * 256].rearrange(
            "p (g s) -> p g s", g=4)[:, :, 0:s]

    def recur_own(q, ps_g):
        s4 = s_pool.tile([128, 512], F16, tag="s", bufs=17)
        shv = s4[:].rearrange("p (g c s) -> p g c s", g=4, c=2)
        gv = ps_g[:].rearrange("p (g c s) -> p g c s", g=4, c=2)
        spv = spre_sb[:, q * 256:(q + 1) * 256].rearrange("p (g s) -> p g s", g=4)
        nc.vector.tensor_copy(shv[:, :, 0, :], spv)
        nc.vector.tensor_mul(shv[:, :, 1, :], spv, a16b(q, 64))
        nc.vector.tensor_add(shv[:, :, 1, :], shv[:, :, 1, :], gv[:, :, 0, :])
        return s4

    sh_tiles = _g_phase(tc, psG, w_p_sb, xt_h[:], recur_own, width=64)

    def recur_v(q, ps_g):
        s4 = s_pool.tile([128, 512], F16, tag="s", bufs=17)
        sv = s4[:].rearrange("p (g c s) -> p g c s", g=4, c=4)
        gv = ps_g[:].rearrange("p (g c s) -> p g c s", g=4, c=4)
        nc.gpsimd.memset(sv[:, :, 0, :], 0.0)
        nc.vector.tensor_copy(sv[:, :, 1, :], gv[:, :, 0, :])
        for cc in (2, 3):
            nc.vector.tensor_mul(sv[:, :, cc, :], sv[:, :, cc - 1, :], a16b(q, 32))
            nc.vector.tensor_add(sv[:, :, cc, :], sv[:, :, cc, :], gv[:, :, cc - 1, :])
        return s4

    sv_tiles = _g_phase(tc, psG, w_p_sb, xt_v[:], recur_v)

    y_h = big_pool.tile([128, NOCT * 128], F16, tag="big", bufs=4, name="y_h")
    _b_phase(tc, psyw, w_t_sb, w_cba_sb, xt_h[:], sh_tiles, y_h)

    proj_pools = (psA, psout, yt_pool, out_pool)
    _proj(tc, proj_pools, wpt_sb, ident[:], y_h[:], aps["zh"])

    y_v = big_pool.tile([128, NOCT * 128], F16, tag="big", bufs=4, name="y_v")
    _b_phase(tc, psyw, w_t_sb, w_cba_sb, xt_v[:], sv_tiles, y_v)
    _proj(tc, proj_pools, wpt_sb, ident[:], y_v[:], aps["zv"])


def build_program(n_cores=8):
    nc = bacc.Bacc("TRN2", target_bir_lowering=False, debug=False,
                   enable_asserts=False, num_devices=n_cores)
    aps = {
        "xtv": nc.dram_tensor("xtv", [128, NOCT * 128], F16, kind="ExternalInput").ap(),
        "xth": nc.dram_tensor("xth", [128, NOCT * 128], F16, kind="ExternalInput").ap(),
        "spre": nc.dram_tensor("spre", [128, NG * W], F16, kind="ExternalInput").ap(),
        "ws": nc.dram_tensor("ws", [128, NG + 1], F32, kind="ExternalInput").ap(),
        "wb1": nc.dram_tensor("wb1", [128, NOCT * 64], F16, kind="ExternalInput").ap(),
        "wbig": nc.dram_tensor("wbig", [128, 16384], F16, kind="ExternalInput").ap(),
        "wpt": nc.dram_tensor("wpt", [128, 2048], F16, kind="ExternalInput").ap(),
        "zv": nc.dram_tensor("zv", [128, Q, D], F16, kind="ExternalOutput").ap(),
        "zh": nc.dram_tensor("zh", [128, Q, D], F16, kind="ExternalOutput").ap(),
    }
    with tile.TileContext(nc) as tc:
        with ExitStack() as ctx:
            _kernel_body(ctx, tc, aps)
    nc.compile()
    return nc


_PROGRAM = None


def _get_program():
    global _PROGRAM
    if _PROGRAM is None:
        _PROGRAM = build_program()
    return _PROGRAM


def make_in_maps(x, A, Bm, Cm, D_skip, Wp):
    WB1, WBIG, WPT2, A16 = _precompute_weights(A, Bm, Cm, D_skip, Wp)
    xg = np.ascontiguousarray(x, dtype=np.float32).reshape(B, H, W, D)
    in_maps = []
    for k in range(8):
        b, half = k // 2, k % 2
        x_own = xg[b, 32 * half:32 * half + 32]
        if half == 0:
            spre = np.zeros((128, NG * W), NP_CDT)
        else:
            spre = _prefix_states(xg[b, 0:32], A)
        ws = np.concatenate(
            [A16, np.full((128, 1), float(half), np.float32)], axis=1)
        in_maps.append({
            "xtv": _pack_v(x_own),
            "xth": _pack_h(x_own),
            "spre": spre,
            "ws": ws, "wb1": WB1, "wbig": WBIG, "wpt": WPT2,
        })
    return in_maps


def assemble_output(results, b_proj):
    out = np.zeros((B, H, W, D), np.float32)
    for k in range(8):
        b, half = k // 2, k % 2
        zv = np.asarray(results[k]["zv"], dtype=np.float32)
        zh = np.asarray(results[k]["zh"], dtype=np.float32)
        yv = zv.reshape(CV, 32, Q, D).transpose(1, 0, 2, 3).reshape(32, W, D)
        yh = zh.reshape(2, W, Q, D).transpose(0, 2, 1, 3).reshape(32, W, D)
        out[b, 32 * half:32 * half + 32] = yv + yh
    out += np.asarray(b_proj, dtype=np.float32)
    return out.reshape(B, H * W, D)


def kernel(x, h, w, A, Bm, Cm, D_skip, Wp, b_proj, **_kw):
    nc = _get_program()
    in_maps = make_in_maps(np.asarray(x), np.asarray(A), np.asarray(Bm),
                           np.asarray(Cm), np.asarray(D_skip), np.asarray(Wp))
    res = run_bass_kernel_spmd(nc, in_maps, list(range(8)))
    return assemble_output(res.results, np.asarray(b_proj))
